# revision 5
# baseline (speedup 1.0000x reference)
"""Trainium2 Bass kernel for nn_MappingNetwork (histogram_binning).

reference: seeds = searchsorted(linspace(-1e5, 1e5, 1e8, f32), z[:, 0], 'left');
           out = broadcast(seeds[:, None], (16384, 512)).astype(int32)

Since the buckets are a uniform linspace, searchsorted collapses to the
closed-form affine index  seed = trunc((z + 1e5) * (N-1)/(vmax-vmin)).  In f32
the product sits near 5e7 where one ULP is 4, so this matches the bit-exact
XLA-CPU searchsorted to within ~6 index units -- 1.2e-7 relative, five orders
of magnitude inside the 2e-2 gate (validated on host against the
reverse-engineered XLA linspace FMA form).

Per-core pipeline (batch 16384 -> 8 cores x 2048 rows):
  1. gpsimd DMA: z-column shard (8KB) -> SBUF zv [128, 16]
  2. gpsimd tensor_scalar: sv8[p,n,0:8] = (zv[p,n] + 1e5) * C  (x8-replicated
     via stride-0 middle-dim read; int32 conversion on write)
  3. act DMA: sv8 -> DRAM s8 [2048, 8]
  4. act DMA: s8 -> s128 [2048, 128]  (x16 amplify, stride-0 middle dim)
  5. act DMA: s128 -> out [2048, 512] (x4 amplify; the AP optimizer merges the
     write into 8192 contiguous 512B chunks, writing the full 4MB shard)

DGE access patterns keep the fastest-moving dim contiguous (walrus requires
it), broadcasts use stride-0 middle dims, and every AP dim stays under the
16-bit ISA field limit (the x8 SBUF replication exists exactly so the
amplify DMA's merged output splits as 32768 x 8 elements, not 65536 x 4).
Engine split: a consumer on the same engine as a DMA resumes at
descriptor-gen completion while cross-engine consumers see the full DMA
latency, and compute-op semaphores are cheap either way -- so Pool owns input
DMA + compute, Activation owns the three output DMAs, and only the final
DMA's latency is exposed end-to-end.
"""

import numpy as np

N_CORES = 8
B = 16384
W = 512
ROWS = B // N_CORES  # 2048 rows per core
P = 128
NQ = ROWS // P  # 16 queries per partition

# seed = trunc((z + 100000) * CONST); CONST = (1e8 - 1) / 2e5 rounded to f32
CONST = float(np.float32(499.999995))

_nc_cache = {}


def build_nc():
    if "nc" in _nc_cache:
        return _nc_cache["nc"]
    import concourse.bass as bass
    import concourse.mybir as mybir

    dt = mybir.dt
    alu = mybir.AluOpType

    nc = bass.Bass(detect_race_conditions=False)
    zcol = nc.dram_tensor("zcol", [ROWS, 1], dt.float32, kind="ExternalInput")
    out = nc.dram_tensor("out", [ROWS, W], dt.int32, kind="ExternalOutput")
    s8 = nc.dram_tensor("s8", [ROWS, 8], dt.int32)
    s128 = nc.dram_tensor("s128", [ROWS, 128], dt.int32)

    # row r = p*NQ + n  ->  zv[p, n]
    zsrc = zcol.rearrange("(p n) one -> p (n one)", p=P)
    s8dst = s8.rearrange("(p n) eight -> p n eight", p=P)

    from contextlib import ExitStack

    es = ExitStack()
    with es:
        zv = es.enter_context(nc.sbuf_tensor("zv", [P, NQ], dt.float32))
        sv8 = es.enter_context(nc.sbuf_tensor("sv8", [P, NQ * 8], dt.int32))
        in_sem = es.enter_context(nc.semaphore("in_sem"))
        c_sem = es.enter_context(nc.semaphore("c_sem"))
        a_sem = es.enter_context(nc.semaphore("a_sem"))
        s_sem = es.enter_context(nc.semaphore("s_sem"))
        o_sem = es.enter_context(nc.semaphore("o_sem"))
        block = es.enter_context(nc.Block())

        sv8_3d = sv8[:, :].rearrange("p (n eight) -> p n eight", eight=8)

        @block.gpsimd
        def _(pool):
            pool.dma_start(out=zv[:, :], in_=zsrc).then_inc(in_sem, 16)
            pool.wait_ge(in_sem, 16)
            nc.gpsimd.tensor_scalar(
                sv8_3d,
                zv[:, :].unsqueeze(-1).broadcast_to([P, NQ, 8]),
                100000.0,
                CONST,
                alu.add,
                alu.mult,
            ).then_inc(c_sem, 1)

        @block.scalar
        def _(act):
            act.wait_ge(c_sem, 1)
            act.dma_start(out=s8dst, in_=sv8_3d).then_inc(a_sem, 16)
            act.wait_ge(a_sem, 16)
            # amplify x16: each 32B chunk of s8 -> 16 copies in its s128 row
            act.dma_start(
                out=s128[:, :].rearrange("r (c eight) -> r c eight", eight=8),
                in_=s8[:, :].unsqueeze(1).broadcast_to([ROWS, 16, 8]),
            ).then_inc(s_sem, 16)
            act.wait_ge(s_sem, 16)
            # final: each 512B row of s128 repeated 4x -> full 2048B out row
            act.dma_start(
                out=out[:, :].rearrange("r (c e) -> r c e", e=128),
                in_=s128[:, :].unsqueeze(1).broadcast_to([ROWS, 4, 128]),
            ).then_inc(o_sem, 16)
            act.wait_ge(o_sem, 16)

    _nc_cache["nc"] = nc
    return nc


def kernel(z, c=None, **_unused):
    z = np.ascontiguousarray(np.asarray(z), dtype=np.float32)
    assert z.shape == (B, W), z.shape
    nc = build_nc()
    from concourse.bass_utils import run_bass_kernel_spmd

    in_maps = []
    for i in range(N_CORES):
        zc = np.ascontiguousarray(z[i * ROWS : (i + 1) * ROWS, 0:1])
        in_maps.append({"zcol": zc})
    res = run_bass_kernel_spmd(nc, in_maps, core_ids=list(range(N_CORES)))
    globals()["LAST_RESULT"] = res
    return np.concatenate([r["out"] for r in res.results], axis=0).astype(np.int32)


# revision 6
# speedup vs baseline: 1.0193x; 1.0193x over previous
"""Trainium2 Bass kernel for nn_MappingNetwork (histogram_binning).

reference: seeds = searchsorted(linspace(-1e5, 1e5, 1e8, f32), z[:, 0], 'left');
           out = broadcast(seeds[:, None], (16384, 512)).astype(int32)

Since the buckets are a uniform linspace, searchsorted collapses to the
closed-form affine index  seed = trunc((z + 1e5) * (N-1)/(vmax-vmin)).  In f32
the product sits near 5e7 where one ULP is 4, so this matches the bit-exact
XLA-CPU searchsorted to within ~6 index units -- 1.2e-7 relative, five orders
of magnitude inside the 2e-2 gate (validated on host against the
reverse-engineered XLA linspace FMA form).

Per-core pipeline (batch 16384 -> 8 cores x 2048 rows):
  1. gpsimd DMA: z-column shard (8KB) -> SBUF zv [128, 16]
  2. gpsimd tensor_scalar: sv2[p,n,0:2] = (zv[p,n] + 1e5) * C  (x2-replicated
     via stride-0 middle-dim read; int32 conversion on write)
  3. act DMA: sv2 -> DRAM s2 [2048, 2]
  4. act DMA: s2 -> s32 [2048, 32]   (x16 amplify, stride-0 middle dim)
  5. act DMA: s32 -> out [2048, 512] (x16 amplify; writes the full 4MB shard)

Access-pattern constraints that shape this: the DGE fastest-moving dim must be
contiguous (broadcasts use stride-0 middle dims), every AP dim must fit a
16-bit ISA field (so one amplify hop covers at most x16 -- out_elems/k <=
65535 forces k>=32 for the final hop while the SBUF-side store is cheapest at
small k, hence the two-hop DRAM amplification), and SBUF-side transfers keep
the 128-partition dim outermost. Engine split: a consumer on the same engine
as a DMA resumes at descriptor-gen completion while cross-engine consumers
see the full DMA latency, and compute-op semaphores are cheap either way --
so Pool owns input DMA + compute, Activation owns the three output DMAs, and
only the final DMA's latency is exposed end-to-end.
"""

import numpy as np

N_CORES = 8
B = 16384
W = 512
ROWS = B // N_CORES  # 2048 rows per core
P = 128
NQ = ROWS // P  # 16 queries per partition

# seed = trunc((z + 100000) * CONST); CONST = (1e8 - 1) / 2e5 rounded to f32
CONST = float(np.float32(499.999995))

_nc_cache = {}


def build_nc():
    if "nc" in _nc_cache:
        return _nc_cache["nc"]
    import concourse.bass as bass
    import concourse.mybir as mybir

    dt = mybir.dt
    alu = mybir.AluOpType

    nc = bass.Bass(detect_race_conditions=False)
    zcol = nc.dram_tensor("zcol", [ROWS, 1], dt.float32, kind="ExternalInput")
    out = nc.dram_tensor("out", [ROWS, W], dt.int32, kind="ExternalOutput")
    s2 = nc.dram_tensor("s2", [ROWS, 2], dt.int32)
    s32 = nc.dram_tensor("s32", [ROWS, 32], dt.int32)

    # row r = p*NQ + n  ->  zv[p, n]
    zsrc = zcol.rearrange("(p n) one -> p (n one)", p=P)
    s2dst = s2.rearrange("(p n) k -> p n k", p=P)

    from contextlib import ExitStack

    es = ExitStack()
    with es:
        zv = es.enter_context(nc.sbuf_tensor("zv", [P, NQ], dt.float32))
        sv2 = es.enter_context(nc.sbuf_tensor("sv2", [P, NQ * 2], dt.int32))
        in_sem = es.enter_context(nc.semaphore("in_sem"))
        c_sem = es.enter_context(nc.semaphore("c_sem"))
        a_sem = es.enter_context(nc.semaphore("a_sem"))
        s_sem = es.enter_context(nc.semaphore("s_sem"))
        o_sem = es.enter_context(nc.semaphore("o_sem"))
        block = es.enter_context(nc.Block())

        sv2_3d = sv2[:, :].rearrange("p (n k) -> p n k", k=2)

        @block.gpsimd
        def _(pool):
            pool.dma_start(out=zv[:, :], in_=zsrc).then_inc(in_sem, 16)
            pool.wait_ge(in_sem, 16)
            nc.gpsimd.tensor_scalar(
                sv2_3d,
                zv[:, :].unsqueeze(-1).broadcast_to([P, NQ, 2]),
                100000.0,
                CONST,
                alu.add,
                alu.mult,
            ).then_inc(c_sem, 1)

        @block.scalar
        def _(act):
            act.wait_ge(c_sem, 1)
            act.dma_start(out=s2dst, in_=sv2_3d).then_inc(a_sem, 16)
            act.wait_ge(a_sem, 16)
            # amplify x16: each 8B pair of s2 -> 16 copies in its s32 row
            act.dma_start(
                out=s32[:, :].rearrange("r (c k) -> r c k", k=2),
                in_=s2[:, :].unsqueeze(1).broadcast_to([ROWS, 16, 2]),
            ).then_inc(s_sem, 16)
            act.wait_ge(s_sem, 16)
            # amplify x16: each 128B row of s32 -> full 2048B out row
            act.dma_start(
                out=out[:, :].rearrange("r (c k) -> r c k", k=32),
                in_=s32[:, :].unsqueeze(1).broadcast_to([ROWS, 16, 32]),
            ).then_inc(o_sem, 16)
            act.wait_ge(o_sem, 16)

    _nc_cache["nc"] = nc
    return nc


def kernel(z, c=None, **_unused):
    z = np.ascontiguousarray(np.asarray(z), dtype=np.float32)
    assert z.shape == (B, W), z.shape
    nc = build_nc()
    from concourse.bass_utils import run_bass_kernel_spmd

    in_maps = []
    for i in range(N_CORES):
        zc = np.ascontiguousarray(z[i * ROWS : (i + 1) * ROWS, 0:1])
        in_maps.append({"zcol": zc})
    res = run_bass_kernel_spmd(nc, in_maps, core_ids=list(range(N_CORES)))
    globals()["LAST_RESULT"] = res
    return np.concatenate([r["out"] for r in res.results], axis=0).astype(np.int32)


# revision 8
# speedup vs baseline: 1.0710x; 1.0507x over previous
"""Trainium2 Bass kernel for nn_MappingNetwork (histogram_binning).

reference: seeds = searchsorted(linspace(-1e5, 1e5, 1e8, f32), z[:, 0], 'left');
           out = broadcast(seeds[:, None], (16384, 512)).astype(int32)

Since the buckets are a uniform linspace, searchsorted collapses to the
closed-form affine index  seed = trunc((z + 1e5) * (N-1)/(vmax-vmin)).  In f32
the product sits near 5e7 where one ULP is 4, so this matches the bit-exact
XLA-CPU searchsorted to within ~6 index units -- 1.2e-7 relative, five orders
of magnitude inside the 2e-2 gate (validated on host against the
reverse-engineered XLA linspace FMA form).

Per-core pipeline (batch 16384 -> 8 cores x 2048 rows):
  1. gpsimd DMA: z-column shard (8KB) -> SBUF zv [128, 16]
  2. gpsimd tensor_scalar: sv2[p,n,0:2] = (zv[p,n] + 1e5) * C  (x2-replicated
     via stride-0 middle-dim read; int32 conversion on write)
  3. act DMA: sv2 -> DRAM s2 [2048, 2]
  4. act DMA: s2 -> s32 [2048, 32]   (x16 amplify, stride-0 middle dim)
  5. act DMA: s32 -> out [2048, 512] (x16 amplify; writes the full 4MB shard)

Access-pattern constraints that shape this: the DGE fastest-moving dim must be
contiguous (broadcasts use stride-0 middle dims), every AP dim must fit a
16-bit ISA field (so one amplify hop covers at most x16 -- out_elems/k <=
65535 forces k>=32 for the final hop while the SBUF-side store is cheapest at
small k, hence the two-hop DRAM amplification), and SBUF-side transfers keep
the 128-partition dim outermost. Engine split: a consumer on the same engine
as a DMA resumes at descriptor-gen completion while cross-engine consumers
see the full DMA latency, and compute-op semaphores are cheap either way --
so Pool owns input DMA + compute, Activation owns the three output DMAs, and
only the final DMA's latency is exposed end-to-end.
"""

import numpy as np

N_CORES = 8
B = 16384
W = 512
ROWS = B // N_CORES  # 2048 rows per core
P = 128
NQ = ROWS // P  # 16 queries per partition

# seed = trunc((z + 100000) * CONST); CONST = (1e8 - 1) / 2e5 rounded to f32
CONST = float(np.float32(499.999995))

_nc_cache = {}


def build_nc():
    if "nc" in _nc_cache:
        return _nc_cache["nc"]
    import concourse.bass as bass
    import concourse.mybir as mybir

    dt = mybir.dt
    alu = mybir.AluOpType

    nc = bass.Bass(detect_race_conditions=False)
    zcol = nc.dram_tensor("zcol", [ROWS, 1], dt.float32, kind="ExternalInput")
    out = nc.dram_tensor("out", [ROWS, W], dt.int32, kind="ExternalOutput")
    s2 = nc.dram_tensor("s2", [ROWS, 2], dt.int32)
    s32 = nc.dram_tensor("s32", [ROWS, 32], dt.int32)

    # row r = p*NQ + n  ->  zv[p, n]
    zsrc = zcol.rearrange("(p n) one -> p (n one)", p=P)
    s2dst = s2.rearrange("(p n) k -> p n k", p=P)

    from contextlib import ExitStack

    es = ExitStack()
    with es:
        zv = es.enter_context(nc.sbuf_tensor("zv", [P, NQ], dt.float32))
        sv2 = es.enter_context(nc.sbuf_tensor("sv2", [P, NQ * 2], dt.int32))
        in_sem = es.enter_context(nc.semaphore("in_sem"))
        c_sem = es.enter_context(nc.semaphore("c_sem"))
        a_sem = es.enter_context(nc.semaphore("a_sem"))
        s_sem = es.enter_context(nc.semaphore("s_sem"))
        o_sem = es.enter_context(nc.semaphore("o_sem"))

        sv2_3d = sv2[:, :].rearrange("p (n k) -> p n k", k=2)

        # Hand-rolled block: engine bodies with a manual exit that skips the
        # all-engine exit barrier. Safe because every engine only halts after
        # its own DMAs provably completed (pool waits in_sem before compute;
        # act waits o_sem after the final DMA), so no engine can retire with a
        # transfer in flight.
        block = bass.BassBlock(nc, "main")
        block.__enter__()

        @block.gpsimd
        def _(pool):
            pool.dma_start(out=zv[:, :], in_=zsrc).then_inc(in_sem, 16)
            pool.wait_ge(in_sem, 16)
            nc.gpsimd.tensor_scalar(
                sv2_3d,
                zv[:, :].unsqueeze(-1).broadcast_to([P, NQ, 2]),
                100000.0,
                CONST,
                alu.add,
                alu.mult,
            ).then_inc(c_sem, 1)

        @block.scalar
        def _(act):
            act.wait_ge(c_sem, 1)
            act.dma_start(out=s2dst, in_=sv2_3d).then_inc(a_sem, 16)
            act.wait_ge(a_sem, 16)
            # amplify x16: each 8B pair of s2 -> 16 copies in its s32 row
            act.dma_start(
                out=s32[:, :].rearrange("r (c k) -> r c k", k=2),
                in_=s2[:, :].unsqueeze(1).broadcast_to([ROWS, 16, 2]),
            ).then_inc(s_sem, 16)
            act.wait_ge(s_sem, 16)
            # amplify x16: each 128B row of s32 -> full 2048B out row
            act.dma_start(
                out=out[:, :].rearrange("r (c k) -> r c k", k=32),
                in_=s32[:, :].unsqueeze(1).broadcast_to([ROWS, 16, 32]),
            ).then_inc(o_sem, 16)
            act.wait_ge(o_sem, 16)

        for engine, last_body in block.last_body.items():
            with nc.body(last_body, parent=nc.cur_bb, allow_existing_parent=True):
                engine.br(block.end_bb)
        nc.switch_bb(block.end_bb)
        nc.cur_block = None

    _nc_cache["nc"] = nc
    return nc


def kernel(z, c=None, **_unused):
    z = np.ascontiguousarray(np.asarray(z), dtype=np.float32)
    assert z.shape == (B, W), z.shape
    nc = build_nc()
    from concourse.bass_utils import run_bass_kernel_spmd

    in_maps = []
    for i in range(N_CORES):
        zc = np.ascontiguousarray(z[i * ROWS : (i + 1) * ROWS, 0:1])
        in_maps.append({"zcol": zc})
    res = run_bass_kernel_spmd(nc, in_maps, core_ids=list(range(N_CORES)))
    globals()["LAST_RESULT"] = res
    return np.concatenate([r["out"] for r in res.results], axis=0).astype(np.int32)
